# revision 1
# baseline (speedup 1.0000x reference)
"""Multi-head attention (B=4, S=2048, D=1024, H=16) on 8 Trainium2 cores.

Sharding (v6): core c -> head-pair p = c (2 heads, 128 output dims), all 4
batches.  This makes the valid_len truncation SPMD-uniform: every core runs
the same per-batch k-loop trip counts kc_b = ceil(valid_len[b]/128), so the
~50% of attention work beyond the key-padding boundary is simply never
emitted.  W_o is row-split by head-pair; each core emits a full-shape
[B, S, D] fp16 partial and the host sums the 8 partials.

Masking: the host zeroes xv columns at k >= valid_len[b] and supplies a
masked ones-column, so masked keys contribute exactly 0 to both the
attention*V accumulation and the softmax denominator.  exp then needs no
bias at all (scores at masked-but-computed boundary columns are finite).

v6 scheduling changes over v5 (which ran at 423-475 us):
  - All input DMA chunked to [128, <=512] tiles (v5 loaded xv as one
    [128, 2048] tile per dj; its 4 MB DMA head-of-line-blocked the PE
    queue behind the V-projection matmuls).
  - Per batch, K/V projections are emitted block-by-block in the order
    attention consumes them, with the Q projection for query block qb
    spread between K/V blocks, so the DMA stream mirrors consumption.
  - O-projection is interleaved per query block (one qb behind the
    attention pipeline) instead of running after the whole batch; this
    removes the serial tail and gives the PE fill work mid-attention.
  - av PSUM evacuation is a single DVE copy (v5 split it DVE+ACT; the
    ACT half contended with the next exp and delayed the PSUM release).
"""

import contextlib

import numpy as np
import ml_dtypes

import concourse.bacc as bacc
import concourse.mybir as mybir
import concourse.tile as tile
from concourse.bass_utils import run_bass_kernel_spmd

BF16 = mybir.dt.bfloat16
F16 = mybir.dt.float16
F32 = mybir.dt.float32
AF = mybir.ActivationFunctionType

B, S, D, H, HD = 4, 2048, 1024, 16, 64
NQB = S // 512        # query blocks of 512

_cache = {}


class _Fill:
    """Round-robin over the two single-bank fill-PSUM slots (tags pqk/pv)
    so consecutive fill matmul groups double-buffer across banks."""

    def __init__(self, pool):
        self.pool = pool
        self.i = 0

    def tile(self, shape, name):
        tg = "pqk" if self.i % 2 == 0 else "pv"
        self.i += 1
        return self.pool.tile(shape, F32, tag=tg, name=name)


def _stream_in(nc, stream, ap, b, kind, dj, c0, n):
    t = stream.tile([128, 512], BF16, tag=kind, name=f"{kind}t", bufs=16)
    nc.sync.dma_start(
        t[:, 0:n], ap[f"{kind}{b}"][dj * 128:(dj + 1) * 128, c0:c0 + n])
    return t


def _q_proj_qb(nc, b, qb, ap, stream, fill, wq_sb, qT_sb, tiles=None):
    psq = fill.tile([128, 512], "psq")
    for dj in range(8):
        xqt = tiles[dj] if tiles else _stream_in(
            nc, stream, ap, b, "xq", dj, qb * 512, 512)
        nc.tensor.matmul(psq[:], wq_sb[dj][:], xqt[:],
                         start=(dj == 0), stop=(dj == 7))
    nc.vector.tensor_copy(qT_sb[b][:, qb * 512:(qb + 1) * 512], psq[:])


def _k_proj_blk(nc, b, blk, n, ap, stream, fill, wk_sb, kT_sb, tiles=None):
    psk = fill.tile([128, 512], "psk")
    for dj in range(8):
        xkt = tiles[dj] if tiles else _stream_in(
            nc, stream, ap, b, "xk", dj, blk * 512, n)
        nc.tensor.matmul(psk[:, 0:n], wk_sb[dj][:], xkt[:, 0:n],
                         start=(dj == 0), stop=(dj == 7))
    nc.vector.tensor_copy(
        kT_sb[b][:, blk * 512:blk * 512 + n], psk[:, 0:n])


def _v_proj_blk(nc, b, blk, n, ap, stream, fill, wv_sb, v_sb, vm_sb,
                tiles=None):
    """V projection for one 512-key block: natural [s, dout] layout plus a
    host-masked ones column per head.  v_sb[b][sc] is [128, 2, HD+1]."""
    xv_sb = tiles or [_stream_in(nc, stream, ap, b, "xv", dj, blk * 512, n)
                      for dj in range(8)]
    for sl in range(n // 128):
        sc = blk * 4 + sl
        psv = fill.tile([128, 2, HD], "psv")
        for dj in range(8):
            nc.tensor.matmul(
                psv[:], xv_sb[dj][:, sl * 128:(sl + 1) * 128],
                wv_sb[dj][:], start=(dj == 0), stop=(dj == 7))
        nc.vector.tensor_copy(v_sb[b][sc][:, :, 0:HD], psv[:])
        nc.vector.tensor_copy(
            v_sb[b][sc][:, :, HD], vm_sb[:, b * 16 + sc, :])


def _fill_units(nc, b, kcb, ap, stream, fill, wq_sb, wk_sb, wv_sb,
                qT_sb, kT_sb, v_sb, vm_sb, eager_dma=False):
    """Yield per-unit emission closures for one batch's projections, in
    attention-consumption order: Q(qb) spread between K/V key blocks.
    Each unit is ~8 matmuls so it can be interleaved between attention
    steps of the previous batch at fine grain.

    With eager_dma, each unit's dma_starts are emitted immediately (so the
    next batch's input stream is issued up-front, decoupled from the pace
    of the previous batch's attention) and the closure only emits the
    matmuls+copy."""
    nk = kcb * 128
    nblk = (nk + 511) // 512

    def q_unit(qb):
        t = [_stream_in(nc, stream, ap, b, "xq", dj, qb * 512, 512)
             for dj in range(8)] if eager_dma else None
        return lambda: _q_proj_qb(nc, b, qb, ap, stream, fill, wq_sb,
                                  qT_sb, t)

    def k_unit(blk, n):
        t = [_stream_in(nc, stream, ap, b, "xk", dj, blk * 512, n)
             for dj in range(8)] if eager_dma else None
        return lambda: _k_proj_blk(nc, b, blk, n, ap, stream, fill,
                                   wk_sb, kT_sb, t)

    def v_unit(blk, n):
        t = [_stream_in(nc, stream, ap, b, "xv", dj, blk * 512, n)
             for dj in range(8)] if eager_dma else None
        return lambda: _v_proj_blk(nc, b, blk, n, ap, stream, fill,
                                   wv_sb, v_sb, vm_sb, t)

    for blk in range(max(nblk, NQB)):
        if blk < NQB:
            yield q_unit(blk)
        if blk < nblk:
            n = min(512, nk - blk * 512)
            yield k_unit(blk, n)
            yield v_unit(blk, n)


def _fills_batch(nc, b, kcb, ap, stream, fill, wq_sb, wk_sb, wv_sb,
                 qT_sb, kT_sb, v_sb, vm_sb):
    for u in _fill_units(nc, b, kcb, ap, stream, fill, wq_sb, wk_sb,
                         wv_sb, qT_sb, kT_sb, v_sb, vm_sb):
        u()


def _attn_qb(nc, b, qb, kcb, psum, expool, wrk, qT_sb, kT_sb, v_sb, ctx_sb,
             filler=None, steps_left=None):
    """Attention for one query block (2 heads): ACT-paced (qb, kc) pipeline.
    After each kc step, pops fill-work units (next batch's projections)
    from `filler` so their matmuls land between attention matmuls in the
    engine FIFOs."""
    av = psum.tile([HD + 1, 2, 512], F32, tag="av", name="av")
    for kc in range(kcb):
        scp = psum.tile([128, 2, 512], F32, tag="sc", name="scp", bufs=2)
        for h2 in range(2):
            nc.tensor.matmul(
                scp[:, h2, :],
                kT_sb[b][64 * h2:64 * h2 + 64, kc * 128:(kc + 1) * 128],
                qT_sb[b][64 * h2:64 * h2 + 64, qb * 512:(qb + 1) * 512],
                start=True, stop=True)
        ex = expool.tile([128, 2, 512], BF16, tag="ex", name="ex")
        nc.scalar.activation(ex[:], scp[:], AF.Exp, scale=0.125)
        for h2 in range(2):
            nc.tensor.matmul(
                av[:, h2, :], v_sb[b][kc][:, h2, :], ex[:, h2, :],
                start=(kc == 0), stop=(kc == kcb - 1))
        if filler and steps_left is not None:
            # Spread remaining fill units evenly over remaining steps.
            want = -(-len(filler) // max(steps_left[0], 1))
            for _ in range(min(want, len(filler))):
                filler.popleft()()
            steps_left[0] -= 1
    # Evacuate av to SBUF in one DVE op (frees the PSUM slot), then
    # normalize off the critical path: ctx[m, q] = av[m, q] / av[64, q].
    avc = wrk.tile([HD + 1, 2, 512], F32, tag="avc", name="avc")
    nc.vector.tensor_copy(avc[:], av[:])
    # Broadcast the sums row to partitions 0..63, then approx-recip there
    # (reciprocal_approx_fast breaks on 1-partition APs, and
    # partition_broadcast only reads partition 0 of its source).
    r0 = wrk.tile([1, 1024], F32, tag="r0", name="r0")
    # gpsimd-issued: keeps attention-critical DMAs off the sync queue,
    # which is busy issuing the bulk input streams.
    nc.gpsimd.dma_start(r0[:], avc[HD:HD + 1, :, :])
    bc = wrk.tile([HD, 1024], F32, tag="bc", name="bc")
    nc.gpsimd.partition_broadcast(bc[:], r0[0:1, :])
    recb = wrk.tile([HD, 1024], F32, tag="recb", name="recb")
    nc.vector.reciprocal_approx_fast(recb[:], bc[:])
    nc.vector.tensor_mul(
        ctx_sb[b][0:HD, qb * 512:(qb + 1) * 512],
        avc[0:HD, 0, :], recb[:, 0:512])
    tmp = wrk.tile([HD, 512], BF16, tag="tmpb", name="tmp")
    nc.vector.tensor_mul(tmp[:], avc[0:HD, 1, :], recb[:, 512:1024])
    nc.gpsimd.dma_start(
        ctx_sb[b][HD:128, qb * 512:(qb + 1) * 512], tmp[:])


def _o_proj_qb(nc, b, qb, ap, fill, wrk, ctx_sb, wo_sb):
    """Output projection partial for one query block (4 sc chunks x 2
    halves); alternating fill-PSUM slots pipeline it 2-deep."""
    for sl in range(4):
        sc = qb * 4 + sl
        for ih in range(2):
            po = fill.tile([128, 512], "po")
            nc.tensor.matmul(
                po[:], ctx_sb[b][:, sc * 128:(sc + 1) * 128],
                wo_sb[0][:, ih * 512:(ih + 1) * 512],
                start=True, stop=True)
            ot = wrk.tile([128, 512], F16, tag="ot", name="ot", bufs=4)
            if ih == 0:
                nc.vector.tensor_copy(ot[:], po[:])
            else:
                nc.scalar.activation(ot[:], po[:], AF.Copy)
            # sync-issued (HWDGE): bulk output writes are too slow through
            # gpsimd's SWDGE path; only the small latency-critical r0/tmp
            # DMAs stay on gpsimd.
            nc.sync.dma_start(
                ap["out"][b, sc * 128:(sc + 1) * 128,
                          ih * 512:(ih + 1) * 512], ot[:])


def _emit(nc, tc, ap, kcs):
    es = contextlib.ExitStack()
    with es:
        const = es.enter_context(tc.tile_pool(name="const", bufs=1))
        resid = es.enter_context(tc.tile_pool(name="resid", bufs=1))
        stream = es.enter_context(tc.tile_pool(name="stream", bufs=3))
        expool = es.enter_context(tc.tile_pool(name="expool", bufs=3))
        wrk = es.enter_context(tc.tile_pool(name="wrk", bufs=2))

        # constants: per-dj [din-chunk, dout=128] weight tiles for the pair
        wq_sb = [const.tile([128, 128], BF16, tag=f"wq{i}", name=f"wq{i}")
                 for i in range(8)]
        wk_sb = [const.tile([128, 128], BF16, tag=f"wk{i}", name=f"wk{i}")
                 for i in range(8)]
        wv_sb = [const.tile([128, 2, HD], BF16, tag=f"wv{i}", name=f"wv{i}")
                 for i in range(8)]
        wo_sb = [const.tile([128, D], BF16, tag="wo", name="wo")]
        vm_sb = const.tile([128, 64, 2], BF16, tag="vmask", name="vmask")
        nc.sync.dma_start(vm_sb[:], ap["vones"])
        for i in range(8):
            nc.sync.dma_start(wq_sb[i][:], ap["wq"][i * 128:(i + 1) * 128, :])
            nc.sync.dma_start(wk_sb[i][:], ap["wk"][i * 128:(i + 1) * 128, :])
            nc.sync.dma_start(wv_sb[i][:],
                              ap["wv"][i * 128:(i + 1) * 128, :, :])
        nc.sync.dma_start(wo_sb[0][:], ap["wo"])

        # residents (per batch)
        qT_sb = [resid.tile([128, S], BF16, tag=f"qT{b}", name=f"qT{b}")
                 for b in range(B)]
        kT_sb = [resid.tile([128, kcs[b] * 128], BF16, tag=f"kT{b}",
                            name=f"kT{b}") for b in range(B)]
        ctx_sb = [resid.tile([128, S], BF16, tag=f"ctx{b}", name=f"ctx{b}")
                  for b in range(B)]
        v_sb = [[resid.tile([128, 2, HD + 1], BF16, tag=f"v{b}_{i}",
                            name=f"v{b}_{i}") for i in range(kcs[b])]
                for b in range(B)]

        # Fill pool (2 banks: pqk + pv) carries Q/K/V projections and the
        # O-projection; they run in the PE gaps of the attention pipeline.
        order = sorted(range(B), key=lambda b: -kcs[b])
        with tc.tile_pool(name="fill_psum", bufs=1, space="PSUM") as fp:
            fill = _Fill(fp)
            b0 = order[0]
            _fills_batch(nc, b0, kcs[b0], ap, stream, fill, wq_sb, wk_sb,
                         wv_sb, qT_sb, kT_sb, v_sb, vm_sb)
            with tc.tile_pool(name="at_psum", bufs=1, space="PSUM") as atp:
                # attention first = higher scheduler priority; later
                # batches' projection matmuls are paced one unit every few
                # attention steps so they sit between attention matmuls in
                # the engine FIFOs and soak up the ACT-paced PE slack.
                # Their input DMAs are issued eagerly (slot-recycling is
                # the only throttle), so a data-gated attention phase
                # cannot starve the next batch's prefetch.
                from collections import deque
                for i, b in enumerate(order):
                    if i + 1 < B:
                        nb = order[i + 1]
                        filler = deque(_fill_units(
                            nc, nb, kcs[nb], ap, stream, fill, wq_sb,
                            wk_sb, wv_sb, qT_sb, kT_sb, v_sb, vm_sb))
                    else:
                        filler = deque()
                    steps_left = [NQB * kcs[b]]
                    for qb in range(NQB):
                        _attn_qb(nc, b, qb, kcs[b], atp, expool, wrk,
                                 qT_sb, kT_sb, v_sb, ctx_sb,
                                 filler, steps_left)
                        if qb > 0:
                            _o_proj_qb(nc, b, qb - 1, ap, fill, wrk,
                                       ctx_sb, wo_sb)
                    while filler:
                        filler.popleft()()
                    _o_proj_qb(nc, b, NQB - 1, ap, fill, wrk, ctx_sb, wo_sb)


def _build(kcs):
    key = ("nc", tuple(kcs))
    if key in _cache:
        return _cache[key]
    nc = bacc.Bacc("TRN2", target_bir_lowering=False, debug=False, num_devices=8)
    ap = {"wq": nc.dram_tensor("wq", [D, 128], BF16, kind="ExternalInput").ap(),
          "wk": nc.dram_tensor("wk", [D, 128], BF16, kind="ExternalInput").ap(),
          "wv": nc.dram_tensor("wv", [D, 2, HD], BF16, kind="ExternalInput").ap(),
          "wo": nc.dram_tensor("wo", [128, D], BF16, kind="ExternalInput").ap(),
          "vones": nc.dram_tensor("vones", [128, 64, 2], BF16,
                                  kind="ExternalInput").ap(),
          "out": nc.dram_tensor("out", [B, S, D], F16,
                                kind="ExternalOutput").ap()}
    for b in range(B):
        ap[f"xq{b}"] = nc.dram_tensor(f"xq{b}", [D, S], BF16,
                                      kind="ExternalInput").ap()
        ap[f"xk{b}"] = nc.dram_tensor(f"xk{b}", [D, kcs[b] * 128], BF16,
                                      kind="ExternalInput").ap()
        ap[f"xv{b}"] = nc.dram_tensor(f"xv{b}", [D, kcs[b] * 128], BF16,
                                      kind="ExternalInput").ap()
    with tile.TileContext(nc) as tc:
        _emit(nc, tc, ap, kcs)
    nc.compile()
    _cache[key] = nc
    return nc


def _in_maps(kcs, queries, keys, values, valid_len, W_q, W_k, W_v, W_o):
    bf = ml_dtypes.bfloat16
    # host-masked ones column: 1 where k < valid_len[b], else 0
    # vones[p, b*16+sc, h] = 1 if sc*128+p < valid_len[b] else 0
    kpos = np.arange(16 * 128).reshape(16, 128)
    vones = np.zeros((128, 64, 2), bf)
    for b in range(B):
        v1 = (kpos < int(valid_len[b])).astype(bf)  # [16, 128]
        vones[:, b * 16:(b + 1) * 16, :] = v1.T[:, :, None]
    maps = []
    for c in range(8):
        j0 = 128 * c
        m = {
            "wq": np.ascontiguousarray(W_q[j0:j0 + 128, :].T).astype(bf),
            "wk": np.ascontiguousarray(W_k[j0:j0 + 128, :].T).astype(bf),
            "wv": np.ascontiguousarray(
                W_v[j0:j0 + 128, :].T).astype(bf).reshape(D, 2, HD),
            "wo": np.ascontiguousarray(W_o[:, j0:j0 + 128].T).astype(bf),
            "vones": vones,
        }
        for b in range(B):
            nk = kcs[b] * 128
            xv = values[b][:nk].T.copy()      # [D, nk]
            xv[:, int(valid_len[b]):] = 0.0   # mask padding rows of v
            m[f"xq{b}"] = np.ascontiguousarray(queries[b].T).astype(bf)
            m[f"xk{b}"] = np.ascontiguousarray(keys[b][:nk].T).astype(bf)
            m[f"xv{b}"] = xv.astype(bf)
        maps.append(m)
    return maps


def kernel(queries, keys, values, valid_len, W_q, W_k, W_v, W_o, _run_kwargs=None):
    queries = np.asarray(queries, np.float32)
    keys = np.asarray(keys, np.float32)
    values = np.asarray(values, np.float32)
    valid_len = np.asarray(valid_len)
    W_q = np.asarray(W_q, np.float32)
    W_k = np.asarray(W_k, np.float32)
    W_v = np.asarray(W_v, np.float32)
    W_o = np.asarray(W_o, np.float32)

    kcs = [max(1, min(16, -(-int(valid_len[b]) // 128))) for b in range(B)]
    nc = _build(kcs)
    maps = _in_maps(kcs, queries, keys, values, valid_len, W_q, W_k, W_v, W_o)
    res = run_bass_kernel_spmd(nc, maps, list(range(8)), **(_run_kwargs or {}))
    out = np.zeros((B, S, D), np.float32)
    for c in range(8):
        out += res.results[c]["out"].astype(np.float32)
    if _run_kwargs:
        _cache["last_results"] = res
    return out



# revision 2
# speedup vs baseline: 1.0776x; 1.0776x over previous
"""Multi-head attention (B=4, S=2048, D=1024, H=16) on 8 Trainium2 cores.

Sharding: core c -> head-pair p = c (2 heads, 128 output dims), all 4
batches.  valid_len truncation is SPMD-uniform: every core runs the same
per-batch k-loop trip counts kc_b = ceil(valid_len[b]/128).  W_o is
row-split by head-pair; each core emits a full-shape [B, S, D] fp16
partial and the host sums the 8 partials.

Masking: the host zeroes xv columns at k >= valid_len[b] and supplies a
masked ones-column, so masked keys contribute exactly 0 to both the
attention*V accumulation and the softmax denominator.

v7 (over v6, which ran at ~405-450 us):
  - Per attention step the PE-order is sc(k) -> fill work -> av(k), so
    the projection/O-proj matmuls execute while ACT runs exp(k); v6
    emitted av before the fills, so the in-order PE stream idled on the
    exp semaphore every step (~45% PE idle + HAM clock droop to 1.2GHz).
  - Fill work is queued as ~430-850ns atomic units (Q/K projections in
    column halves, V per 128-key slice, O-proj per 128-row slice) and
    popped against a per-step cycle budget; data-dependency guards pop
    the current batch's own units just-in-time, which also removes the
    serial prefill phase at kernel start.
  - ACT runs softmax exp ONLY.  O-proj PSUM evacuation moved to DVE;
    output DMA issue moved to the scalar HWDGE ring (each [128,n]
    dma_start costs ~590ns of ring issue regardless of n, and the sync
    ring was 72% busy).
  - Input DMA in [128,1024] chunks (4x wider than v6): sync-ring issue
    cost drops from ~280us to ~110us.  Whole batches are prefetched one
    batch ahead; weight loads are interleaved with the first batch's
    chunks in consumption order.
"""

import contextlib
from collections import deque

import numpy as np
import ml_dtypes

import concourse.bacc as bacc
import concourse.mybir as mybir
import concourse.tile as tile
from concourse.bass_utils import run_bass_kernel_spmd

BF16 = mybir.dt.bfloat16
F16 = mybir.dt.float16
F32 = mybir.dt.float32
AF = mybir.ActivationFunctionType

B, S, D, H, HD = 4, 2048, 1024, 16, 64
NQB = S // 512        # query blocks of 512

_cache = {}


class _Fill:
    """Round-robin over the two single-bank fill-PSUM slots so
    consecutive fill matmul groups double-buffer across banks."""

    def __init__(self, pool):
        self.pool = pool
        self.i = 0

    def tile(self, shape, name):
        tg = "pqk" if self.i % 2 == 0 else "pv"
        self.i += 1
        return self.pool.tile(shape, F32, tag=tg, name=name)


def _emit(nc, tc, ap, kcs):
    es = contextlib.ExitStack()
    with es:
        const = es.enter_context(tc.tile_pool(name="const", bufs=1))
        resid = es.enter_context(tc.tile_pool(name="resid", bufs=1))
        stream = es.enter_context(tc.tile_pool(name="stream", bufs=16))
        expool = es.enter_context(tc.tile_pool(name="expool", bufs=3))
        wrk = es.enter_context(tc.tile_pool(name="wrk", bufs=2))

        # constants: per-dj [din-chunk, dout=128] weight tiles for the pair
        wq_sb = [const.tile([128, 128], BF16, tag=f"wq{i}", name=f"wq{i}")
                 for i in range(8)]
        wk_sb = [const.tile([128, 128], BF16, tag=f"wk{i}", name=f"wk{i}")
                 for i in range(8)]
        wv_sb = [const.tile([128, 2, HD], BF16, tag=f"wv{i}", name=f"wv{i}")
                 for i in range(8)]
        wo_sb = const.tile([128, D], BF16, tag="wo", name="wo")
        vm_sb = const.tile([128, 64, 2], BF16, tag="vmask", name="vmask")

        # residents (per batch)
        qT_sb = [resid.tile([128, S], BF16, tag=f"qT{b}", name=f"qT{b}")
                 for b in range(B)]
        kT_sb = [resid.tile([128, kcs[b] * 128], BF16, tag=f"kT{b}",
                            name=f"kT{b}") for b in range(B)]
        ctx_sb = [resid.tile([128, S], BF16, tag=f"ctx{b}", name=f"ctx{b}")
                  for b in range(B)]
        v_sb = [[resid.tile([128, 2, HD + 1], BF16, tag=f"v{b}_{i}",
                            name=f"v{b}_{i}") for i in range(kcs[b])]
                for b in range(B)]

        order = sorted(range(B), key=lambda b: -kcs[b])

        with tc.tile_pool(name="fill_psum", bufs=1, space="PSUM") as fp, \
             tc.tile_pool(name="at_psum", bufs=1, space="PSUM") as atp:
            fill = _Fill(fp)
            filler = deque()      # (pe_cycles, closure)
            done = set()
            chunk_tiles = {}

            def pop_one():
                cyc, fn = filler.popleft()
                fn()
                return cyc

            def pop_budget(budget):
                while filler and budget > 0:
                    budget -= pop_one()

            def pop_until(key):
                while key not in done:
                    pop_one()

            def issue_chunk(b, kind, dj, cj):
                nk = kcs[b] * 128 if kind != "xq" else S
                c0 = cj * 1024
                n = min(1024, nk - c0)
                t = stream.tile([128, 1024], BF16, tag=kind, name=f"{kind}t")
                nc.sync.dma_start(
                    t[:, 0:n], ap[f"{kind}{b}"][dj * 128:(dj + 1) * 128,
                                                c0:c0 + n])
                chunk_tiles[(b, kind, dj, cj)] = t

            def issue_batch_dma(b, weight_cbs=()):
                """Issue a whole batch's input DMAs on the sync ring, in
                consumption order: xq chunk0, per-cj xk/xv, xq chunk1.
                weight_cbs are interleaved weight-load callbacks (first
                batch only)."""
                cbs = list(weight_cbs)
                ncj = (kcs[b] * 128 + 1023) // 1024

                def next_cb():
                    if cbs:
                        cbs.pop(0)()

                next_cb()                       # wq
                for dj in range(8):
                    issue_chunk(b, "xq", dj, 0)
                next_cb()                       # wk
                for cj in range(ncj):
                    for dj in range(8):
                        issue_chunk(b, "xk", dj, cj)
                    if cj == 0:
                        next_cb()               # wv + vones
                    for dj in range(8):
                        issue_chunk(b, "xv", dj, cj)
                for dj in range(8):
                    issue_chunk(b, "xq", dj, 1)
                while cbs:
                    cbs.pop(0)()                # wo + anything left

            def push_q(b, qb):
                cj, co = qb // 2, (qb % 2) * 512

                def half(h):
                    def fn():
                        psq = fill.tile([128, 256], "psq")
                        for dj in range(8):
                            t = chunk_tiles[(b, "xq", dj, cj)]
                            nc.tensor.matmul(
                                psq[:],
                                wq_sb[dj][:],
                                t[:, co + h * 256:co + (h + 1) * 256],
                                start=(dj == 0), stop=(dj == 7))
                        nc.vector.tensor_copy(
                            qT_sb[b][:, qb * 512 + h * 256:
                                     qb * 512 + (h + 1) * 256], psq[:])
                        if h == 1:
                            done.add((b, 'q', qb))
                    return fn
                filler.append((2048, half(0)))
                filler.append((2048, half(1)))

            def push_k(b, blk, n):
                cj = blk // 2
                co = (blk % 2) * 512
                halves = [(h * 256, min(256, n - h * 256))
                          for h in range(2) if n > h * 256]

                def half(idx, off, m):
                    def fn():
                        psk = fill.tile([128, 256], "psk")
                        for dj in range(8):
                            t = chunk_tiles[(b, "xk", dj, cj)]
                            nc.tensor.matmul(
                                psk[:, 0:m], wk_sb[dj][:],
                                t[:, co + off:co + off + m],
                                start=(dj == 0), stop=(dj == 7))
                        nc.vector.tensor_copy(
                            kT_sb[b][:, blk * 512 + off:blk * 512 + off + m],
                            psk[:, 0:m])
                        if idx == len(halves) - 1:
                            done.add((b, 'k', blk))
                    return fn
                for idx, (off, m) in enumerate(halves):
                    filler.append((8 * m, half(idx, off, m)))

            def push_v(b, blk, n):
                cj = blk // 2
                co = (blk % 2) * 512
                nsl = n // 128

                def slice_(sl):
                    def fn():
                        sc = blk * 4 + sl
                        psv = fill.tile([128, 2, HD], "psv")
                        for dj in range(8):
                            t = chunk_tiles[(b, "xv", dj, cj)]
                            nc.tensor.matmul(
                                psv[:],
                                t[:, co + sl * 128:co + (sl + 1) * 128],
                                wv_sb[dj][:], start=(dj == 0), stop=(dj == 7))
                        nc.vector.tensor_copy(v_sb[b][sc][:, :, 0:HD], psv[:])
                        nc.vector.tensor_copy(
                            v_sb[b][sc][:, :, HD], vm_sb[:, b * 16 + sc, :])
                        if sl == nsl - 1:
                            done.add((b, 'v', blk))
                    return fn
                for sl in range(nsl):
                    filler.append((1024, slice_(sl)))

            def push_batch_units(b):
                nk = kcs[b] * 128
                nblk = (nk + 511) // 512
                for blk in range(max(nblk, NQB)):
                    if blk < NQB:
                        push_q(b, blk)
                    if blk < nblk:
                        n = min(512, nk - blk * 512)
                        push_k(b, blk, n)
                        push_v(b, blk, n)

            def push_oproj(b, qb):
                """O-projection partial for one query block: 4 units of
                [128 s-rows x 1024 dout] each (2 matmuls + DVE evac + one
                output DMA on the scalar ring)."""
                def unit(sl):
                    def fn():
                        sc = qb * 4 + sl
                        ot = wrk.tile([128, D], F16, tag="ot", name="ot",
                                      bufs=6)
                        for ih in range(2):
                            po = fill.tile([128, 512], "po")
                            nc.tensor.matmul(
                                po[:], ctx_sb[b][:, sc * 128:(sc + 1) * 128],
                                wo_sb[:, ih * 512:(ih + 1) * 512],
                                start=True, stop=True)
                            nc.vector.tensor_copy(
                                ot[:, ih * 512:(ih + 1) * 512], po[:])
                        nc.scalar.dma_start(
                            ap["out"][b, sc * 128:(sc + 1) * 128, :], ot[:])
                    return fn
                for sl in range(4):
                    filler.append((1024, unit(sl)))

            # ---- weight loads (interleaved with first batch's stream) --
            def load_wq():
                for i in range(8):
                    nc.sync.dma_start(wq_sb[i][:],
                                      ap["wq"][i * 128:(i + 1) * 128, :])

            def load_wk():
                for i in range(8):
                    nc.sync.dma_start(wk_sb[i][:],
                                      ap["wk"][i * 128:(i + 1) * 128, :])

            def load_wv():
                for i in range(8):
                    nc.sync.dma_start(wv_sb[i][:],
                                      ap["wv"][i * 128:(i + 1) * 128, :, :])
                nc.sync.dma_start(vm_sb[:], ap["vones"])

            def load_wo():
                nc.sync.dma_start(wo_sb[:], ap["wo"])

            # ---- main schedule ----------------------------------------
            b0 = order[0]
            issue_batch_dma(b0, weight_cbs=[load_wq, load_wk, load_wv,
                                            load_wo])
            push_batch_units(b0)

            pending_o = []
            for i, b in enumerate(order):
                kcb = kcs[b]
                if i + 1 < B:
                    nb = order[i + 1]
                    issue_batch_dma(nb)
                    push_batch_units(nb)
                for qb in range(NQB):
                    pop_until((b, 'q', qb))
                    av = atp.tile([HD + 1, 2, 512], F32, tag="av", name="av")
                    for kc in range(kcb):
                        blk = kc >> 2
                        pop_until((b, 'k', blk))
                        pop_until((b, 'v', blk))
                        scp = atp.tile([128, 2, 512], F32, tag="sc",
                                       name="scp", bufs=2)
                        for h2 in range(2):
                            nc.tensor.matmul(
                                scp[:, h2, :],
                                kT_sb[b][64 * h2:64 * h2 + 64,
                                         kc * 128:(kc + 1) * 128],
                                qT_sb[b][64 * h2:64 * h2 + 64,
                                        qb * 512:(qb + 1) * 512],
                                start=True, stop=True)
                        ex = expool.tile([128, 2, 512], BF16, tag="ex",
                                         name="ex")
                        nc.scalar.activation(ex[:], scp[:], AF.Exp,
                                             scale=0.125)
                        if kc == min(2, kcb - 1) and pending_o:
                            for p in pending_o:
                                p()
                            pending_o = []
                        # fill work lands here, between sc(k) and av(k),
                        # so the PE chews projections while ACT runs exp.
                        pop_budget(1536)
                        for h2 in range(2):
                            nc.tensor.matmul(
                                av[:, h2, :], v_sb[b][kc][:, h2, :],
                                ex[:, h2, :],
                                start=(kc == 0), stop=(kc == kcb - 1))
                    # Evacuate av (frees the PSUM slot), then normalize off
                    # the critical path: ctx[m, q] = av[m, q] / av[64, q].
                    avc = wrk.tile([HD + 1, 2, 512], F32, tag="avc",
                                   name="avc")
                    nc.vector.tensor_copy(avc[:], av[:])
                    r0 = wrk.tile([1, 1024], F32, tag="r0", name="r0",
                                  bufs=1)
                    nc.gpsimd.dma_start(r0[:], avc[HD:HD + 1, :, :])
                    bc = wrk.tile([HD, 1024], F32, tag="bc", name="bc",
                                  bufs=1)
                    nc.gpsimd.partition_broadcast(bc[:], r0[0:1, :])
                    recb = wrk.tile([HD, 1024], F32, tag="recb", name="recb",
                                    bufs=1)
                    nc.vector.reciprocal_approx_fast(recb[:], bc[:])
                    nc.vector.tensor_mul(
                        ctx_sb[b][0:HD, qb * 512:(qb + 1) * 512],
                        avc[0:HD, 0, :], recb[:, 0:512])
                    tmp = wrk.tile([HD, 512], BF16, tag="tmpb", name="tmp")
                    nc.vector.tensor_mul(tmp[:], avc[0:HD, 1, :],
                                         recb[:, 512:1024])
                    nc.gpsimd.dma_start(
                        ctx_sb[b][HD:128, qb * 512:(qb + 1) * 512], tmp[:])
                    pending_o.append(
                        (lambda bb, qq: (lambda: push_oproj(bb, qq)))(b, qb))
                    pop_budget(2048)
            for p in pending_o:
                p()
            while filler:
                pop_one()


def _build(kcs):
    key = ("nc", tuple(kcs))
    if key in _cache:
        return _cache[key]
    nc = bacc.Bacc("TRN2", target_bir_lowering=False, debug=False,
                   num_devices=8)
    ap = {"wq": nc.dram_tensor("wq", [D, 128], BF16, kind="ExternalInput").ap(),
          "wk": nc.dram_tensor("wk", [D, 128], BF16, kind="ExternalInput").ap(),
          "wv": nc.dram_tensor("wv", [D, 2, HD], BF16, kind="ExternalInput").ap(),
          "wo": nc.dram_tensor("wo", [128, D], BF16, kind="ExternalInput").ap(),
          "vones": nc.dram_tensor("vones", [128, 64, 2], BF16,
                                  kind="ExternalInput").ap(),
          "out": nc.dram_tensor("out", [B, S, D], F16,
                                kind="ExternalOutput").ap()}
    for b in range(B):
        ap[f"xq{b}"] = nc.dram_tensor(f"xq{b}", [D, S], BF16,
                                      kind="ExternalInput").ap()
        ap[f"xk{b}"] = nc.dram_tensor(f"xk{b}", [D, kcs[b] * 128], BF16,
                                      kind="ExternalInput").ap()
        ap[f"xv{b}"] = nc.dram_tensor(f"xv{b}", [D, kcs[b] * 128], BF16,
                                      kind="ExternalInput").ap()
    with tile.TileContext(nc) as tc:
        _emit(nc, tc, ap, kcs)
    nc.compile()
    _cache[key] = nc
    return nc


def _in_maps(kcs, queries, keys, values, valid_len, W_q, W_k, W_v, W_o):
    bf = ml_dtypes.bfloat16
    # host-masked ones column: 1 where k < valid_len[b], else 0
    # vones[p, b*16+sc, h] = 1 if sc*128+p < valid_len[b] else 0
    kpos = np.arange(16 * 128).reshape(16, 128)
    vones = np.zeros((128, 64, 2), bf)
    for b in range(B):
        v1 = (kpos < int(valid_len[b])).astype(bf)  # [16, 128]
        vones[:, b * 16:(b + 1) * 16, :] = v1.T[:, :, None]
    maps = []
    for c in range(8):
        j0 = 128 * c
        m = {
            "wq": np.ascontiguousarray(W_q[j0:j0 + 128, :].T).astype(bf),
            "wk": np.ascontiguousarray(W_k[j0:j0 + 128, :].T).astype(bf),
            "wv": np.ascontiguousarray(
                W_v[j0:j0 + 128, :].T).astype(bf).reshape(D, 2, HD),
            "wo": np.ascontiguousarray(W_o[:, j0:j0 + 128].T).astype(bf),
            "vones": vones,
        }
        for b in range(B):
            nk = kcs[b] * 128
            xv = values[b][:nk].T.copy()      # [D, nk]
            xv[:, int(valid_len[b]):] = 0.0   # mask padding rows of v
            m[f"xq{b}"] = np.ascontiguousarray(queries[b].T).astype(bf)
            m[f"xk{b}"] = np.ascontiguousarray(keys[b][:nk].T).astype(bf)
            m[f"xv{b}"] = xv.astype(bf)
        maps.append(m)
    return maps


def kernel(queries, keys, values, valid_len, W_q, W_k, W_v, W_o,
           _run_kwargs=None):
    queries = np.asarray(queries, np.float32)
    keys = np.asarray(keys, np.float32)
    values = np.asarray(values, np.float32)
    valid_len = np.asarray(valid_len)
    W_q = np.asarray(W_q, np.float32)
    W_k = np.asarray(W_k, np.float32)
    W_v = np.asarray(W_v, np.float32)
    W_o = np.asarray(W_o, np.float32)

    kcs = [max(1, min(16, -(-int(valid_len[b]) // 128))) for b in range(B)]
    nc = _build(kcs)
    maps = _in_maps(kcs, queries, keys, values, valid_len, W_q, W_k, W_v, W_o)
    res = run_bass_kernel_spmd(nc, maps, list(range(8)), **(_run_kwargs or {}))
    out = np.zeros((B, S, D), np.float32)
    for c in range(8):
        out += res.results[c]["out"].astype(np.float32)
    if _run_kwargs:
        _cache["last_results"] = res
    return out
